# revision 2
# baseline (speedup 1.0000x reference)
"""Chunked attention kernel for Trainium2 (Bass/Tile), SPMD over 8 NeuronCores.

Problem (hardcoded):
  x: [B=8, C=1024, L=4096] fp32, Wq/Wk/Wv/Wo: [1024,1024] fp32 (stored [in,out]),
  biases [1024] fp32.  H=8 heads, head_dim=128, CHUNK=64 (block-diagonal attention).
  out = transpose(softmax((xt@Wq)(xt@Wk)^T/sqrt(128) blockwise) @ (xt@Wv) @ Wo, [B,C,L])

Sharding: data-parallel over B — one batch per core. No collectives.

Per-core dataflow (all matmuls fp16 in / fp32 PSUM accumulate):
  Q^T[c,l] = matmul(lhsT=Wq, rhs=x)        (feature-major, x arrives [C,L] = ready)
  K^T[c,l] = matmul(lhsT=Wk, rhs=x)
  V[l,c]   = matmul(lhsT=x,  rhs=Wv)       (token-major)
  per head h, chunk-pair p (128 tokens):
    S^T[k,q] = matmul(lhsT=K^T block, rhs=Q^T block)   (scores transposed)
    E = exp(S^T/sqrt(128)) on the two diagonal 64x64 blocks (ACT), rest zero
  softmax denominators entirely off the PE: partition_all_reduce(add) over E's
  k-partitions on the (otherwise idle) Pool engine gives D[q] replicated on all
  partitions; DVE reciprocal -> R; the normalize folds into the PV eviction
  (tensor_mul instead of tensor_copy).
    P'^T[d,q] = matmul(lhsT=V block, rhs=E)    (unnormalized)
    P^T = P'^T * R                              (DVE, at eviction)
  out^T[c,l] = matmul(lhsT=Wo, rhs=P^T)    -> exactly the [C,L] output layout

Strip 0 streams against the weight DMAs: V-proj then Q-proj run k-tile-OUTER
with 8 simultaneous PSUM accumulations, so each arriving (x[j], w[j]) tile pair
unlocks ~1.7us of PE work (> its ~1.2us DMA time); K runs head-major once all
wk tiles have landed.  The final output m-tile is split into 128-col chunks to
shrink the end-of-kernel evict+DMA tail.
"""

import numpy as np
from contextlib import ExitStack

import concourse.bass as bass
import concourse.bacc as bacc
import concourse.bass_isa as bass_isa
import concourse.tile as tile
import concourse.mybir as mybir

B, C, L = 8, 1024, 4096
H, HD, CHUNK, PAIR = 8, 128, 64, 128
N_CORES = 8
KT = C // 128          # 8 contraction tiles
LT = 512               # tokens per strip
F16 = mybir.dt.float16
F32 = mybir.dt.float32
SCALE = 1.0 / float(np.sqrt(HD))
WNAMES = ("wq", "wk", "wv", "wo")


def _emit(ctx, tc, x_d, w_d, o_d, l_total):
    nc = tc.nc
    NS = l_total // LT     # strips
    NP = LT // PAIR        # chunk-pairs (= token 128-tiles) per strip

    wpool = ctx.enter_context(tc.tile_pool(name="w", bufs=1))
    cpool = ctx.enter_context(tc.tile_pool(name="const", bufs=1))
    xpool = ctx.enter_context(tc.tile_pool(name="xp", bufs=2))
    qpool = ctx.enter_context(tc.tile_pool(name="qp", bufs=2))
    vpool = ctx.enter_context(tc.tile_pool(name="vp", bufs=2))
    epool = ctx.enter_context(tc.tile_pool(name="ep", bufs=1))
    dpool = ctx.enter_context(tc.tile_pool(name="dp", bufs=1))
    rpool = ctx.enter_context(tc.tile_pool(name="rp", bufs=1))
    ppool = ctx.enter_context(tc.tile_pool(name="pp", bufs=2))
    opool = ctx.enter_context(tc.tile_pool(name="op", bufs=1))
    pjps = ctx.enter_context(tc.tile_pool(name="pj", bufs=4, space="PSUM"))
    scps = ctx.enter_context(tc.tile_pool(name="sc", bufs=2, space="PSUM"))
    pvps = ctx.enter_context(tc.tile_pool(name="pv", bufs=2, space="PSUM"))

    def psum8():
        # 8 simultaneously-open [128,512] fp32 accumulators = all 8 PSUM banks,
        # borrowed across the three pools (their rings are idle at strip 0).
        return ([pjps.tile([128, 512], F32, tag="pj", name="ps8") for _ in range(4)]
                + [scps.tile([128, 512], F32, tag="sc", name="ps8") for _ in range(2)]
                + [pvps.tile([128, 512], F32, tag="pv", name="ps8") for _ in range(2)])

    def o_proj(p_t, ls, fine_tail=False):
        o_t = opool.tile([128, KT * LT], F32, tag="o")
        for m in range(KT):
            if fine_tail and m == KT - 1:
                # last output tile in 128-col chunks: the final evict+DMA tail
                # shrinks from ~[512-col evict + 512-col DMA] to a 128-col one
                for c in range(4):
                    ps = pjps.tile([128, 512], F32, tag="pj")
                    for j in range(KT):
                        nc.tensor.matmul(ps[:, 0:PAIR],
                                         wt[("wo", j)][:, m * 128:(m + 1) * 128],
                                         p_t[:, j * LT + c * PAIR:j * LT + (c + 1) * PAIR],
                                         start=(j == 0), stop=(j == KT - 1))
                    nc.vector.tensor_copy(o_t[:, m * LT + c * PAIR:m * LT + (c + 1) * PAIR],
                                          ps[:, 0:PAIR])
                    nc.sync.dma_start(
                        o_d[m * 128:(m + 1) * 128, ls + c * PAIR:ls + (c + 1) * PAIR],
                        o_t[:, m * LT + c * PAIR:m * LT + (c + 1) * PAIR])
                continue
            ps = pjps.tile([128, 512], F32, tag="pj")
            for j in range(KT):
                nc.tensor.matmul(ps[:, 0:LT],
                                 wt[("wo", j)][:, m * 128:(m + 1) * 128],
                                 p_t[:, j * LT:(j + 1) * LT],
                                 start=(j == 0), stop=(j == KT - 1))
            nc.vector.tensor_copy(o_t[:, m * LT:(m + 1) * LT], ps[:, 0:LT])
            nc.sync.dma_start(o_d[m * 128:(m + 1) * 128, ls:ls + LT],
                              o_t[:, m * LT:(m + 1) * LT])

    def load_x(s):
        x_t = xpool.tile([128, KT * LT], F16, tag="x")
        for j in range(KT):
            nc.sync.dma_start(x_t[:, j * LT:(j + 1) * LT],
                              x_d[j * 128:(j + 1) * 128, s * LT:(s + 1) * LT])
        return x_t

    wt = {}

    def load_w(n, j):
        t = wpool.tile([128, C], F16, tag=f"{n}{j}")
        nc.sync.dma_start(t[:], w_d[n][j * 128:(j + 1) * 128, :])
        wt[(n, j)] = t

    # Startup DMA order follows strip-0 consumption: x0[j]+wv[j] pairs feed the
    # k-tile-outer V projection (one pair unlocks 8x512 matmul cycles), then
    # wq for the k-tile-outer Q pass, then wk, then wo.
    x_t0 = xpool.tile([128, KT * LT], F16, tag="x")
    for j in range(KT):
        nc.sync.dma_start(x_t0[:, j * LT:(j + 1) * LT],
                          x_d[j * 128:(j + 1) * 128, 0:LT])
        load_w("wv", j)
    for j in range(KT):
        load_w("wq", j)
    for j in range(KT):
        load_w("wk", j)
    for j in range(KT):
        load_w("wo", j)

    ones = cpool.tile([128, PAIR], F16, tag="ones")
    nc.vector.memset(ones[:], 1.0)
    # e_t is a single persistent buffer: exps rewrite the diagonal blocks every
    # strip, the off-diagonal stays zero from this one memset (the Pool allreduce
    # sums whole 128-partition columns, so the zeros make the colsum exact).
    e_t = epool.tile([128, H * LT], F16, tag="e")
    nc.gpsimd.memset(e_t[:], 0.0)
    d_rep = dpool.tile([128, H * LT], F32, tag="d")
    r_rep = rpool.tile([128, H * LT], F16, tag="r")

    def score_head(qk_t, h):
        # scores (transposed) for head h, then exp of the diagonal 64x64
        # blocks of every pair -> e_t. One strided ACT per half.
        qb = h * 2 * LT
        kb = h * 2 * LT + LT
        sc = scps.tile([128, LT], F32, tag="sc")
        for p in range(NP):
            nc.tensor.matmul(sc[:, p * PAIR:(p + 1) * PAIR],
                             qk_t[:, kb + p * PAIR:kb + (p + 1) * PAIR],
                             qk_t[:, qb + p * PAIR:qb + (p + 1) * PAIR],
                             start=True, stop=True)
        eh = e_t[:, h * LT:(h + 1) * LT]
        for r0, c0 in ((0, 0), (64, 64)):
            nc.scalar.activation(
                eh[r0:r0 + 64, :].rearrange("a (np c) -> a np c", c=PAIR)[:, :, c0:c0 + 64],
                sc[r0:r0 + 64, :].rearrange("a (np c) -> a np c", c=PAIR)[:, :, c0:c0 + 64],
                mybir.ActivationFunctionType.Exp, scale=SCALE)

    def ar_quarter(g):
        # softmax denominators for heads 2g,2g+1 without touching the PE:
        # colsum over the 128 k-partitions on the Pool engine (result
        # replicated on all partitions), reciprocal on DVE.
        lo, hi = g * 2 * LT, (g + 1) * 2 * LT
        nc.gpsimd.partition_all_reduce(d_rep[:, lo:hi], e_t[:, lo:hi],
                                       channels=128,
                                       reduce_op=bass_isa.ReduceOp.add)
        with nc.allow_low_precision(reason="softmax recip fp16 ample"):
            nc.vector.reciprocal(r_rep[:, lo:hi], d_rep[:, lo:hi])

    def pv_head(v_t, p_t, h):
        ps = pvps.tile([128, NP * PAIR], F32, tag="pv")
        for p in range(NP):
            nc.tensor.matmul(ps[:, p * PAIR:(p + 1) * PAIR],
                             v_t[:, p * C + h * 128:p * C + (h + 1) * 128],
                             e_t[:, h * LT + p * PAIR:h * LT + (p + 1) * PAIR],
                             start=True, stop=True)
        # normalization folded into the eviction
        nc.vector.tensor_mul(p_t[:, h * LT:(h + 1) * LT], ps[:],
                             r_rep[:, h * LT:(h + 1) * LT])

    # ---------------- strip 0: DMA-streaming order ----------------
    x_t = x_t0
    # V projection, k-tile-outer over 8 open psums (4 pairs x 2 col-halves)
    v_t = vpool.tile([128, NP * C], F16, tag="v")
    vps = psum8()
    for j in range(KT):
        for g in range(8):
            p, n2 = divmod(g, 2)
            nc.tensor.matmul(vps[g][:],
                             x_t[:, j * LT + p * 128:j * LT + (p + 1) * 128],
                             wt[("wv", j)][:, n2 * 512:(n2 + 1) * 512],
                             start=(j == 0), stop=(j == KT - 1))
    for g in range(8):
        p, n2 = divmod(g, 2)
        nc.vector.tensor_copy(v_t[:, p * C + n2 * 512:p * C + (n2 + 1) * 512],
                              vps[g][:])
    # Q projection, k-tile-outer, one psum per head
    qk_t = qpool.tile([128, 2 * KT * LT], F16, tag="qk")
    qps = psum8()
    for j in range(KT):
        for h in range(H):
            nc.tensor.matmul(qps[h][:],
                             wt[("wq", j)][:, h * 128:(h + 1) * 128],
                             x_t[:, j * LT:(j + 1) * LT],
                             start=(j == 0), stop=(j == KT - 1))
    for h in range(H):
        nc.vector.tensor_copy(qk_t[:, h * 2 * LT:h * 2 * LT + LT], qps[h][:])
    # K head-major (all wk tiles landed during V+Q), scores, exp, denominators
    for h in range(H):
        kb = h * 2 * LT + LT
        ps = pjps.tile([128, 512], F32, tag="pj")
        for j in range(KT):
            nc.tensor.matmul(ps[:, 0:LT],
                             wt[("wk", j)][:, h * 128:(h + 1) * 128],
                             x_t[:, j * LT:(j + 1) * LT],
                             start=(j == 0), stop=(j == KT - 1))
        nc.vector.tensor_copy(qk_t[:, kb:kb + LT], ps[:, 0:LT])
        score_head(qk_t, h)
        if h % 2 == 1:
            ar_quarter(h // 2)
    x_next = load_x(1) if NS > 1 else None
    p_t = ppool.tile([128, KT * LT], F16, tag="p")
    for h in range(H):
        pv_head(v_t, p_t, h)
    p_prev, ls_prev = p_t, 0

    # ---------------- strips 1..NS-1 ----------------
    for s in range(1, NS):
        ls = s * LT
        x_t = x_next

        qk_t = qpool.tile([128, 2 * KT * LT], F16, tag="qk")
        for h in range(H):
            qb = h * 2 * LT
            kb = h * 2 * LT + LT
            for off, nm in ((qb, "wq"), (kb, "wk")):
                ps = pjps.tile([128, 512], F32, tag="pj")
                for j in range(KT):
                    nc.tensor.matmul(ps[:, 0:LT],
                                     wt[(nm, j)][:, h * 128:(h + 1) * 128],
                                     x_t[:, j * LT:(j + 1) * LT],
                                     start=(j == 0), stop=(j == KT - 1))
                nc.vector.tensor_copy(qk_t[:, off:off + LT], ps[:, 0:LT])
            score_head(qk_t, h)
            if h % 2 == 1:
                ar_quarter(h // 2)

        x_next = load_x(s + 1) if s + 1 < NS else None

        # V projection (token-major): V[l, c] per 128-token tile
        v_t = vpool.tile([128, NP * C], F16, tag="v")
        for p in range(NP):
            for n2 in range(C // 512):
                ps = pjps.tile([128, 512], F32, tag="pj")
                for j in range(KT):
                    nc.tensor.matmul(ps[:],
                                     x_t[:, j * LT + p * 128:j * LT + (p + 1) * 128],
                                     wt[("wv", j)][:, n2 * 512:(n2 + 1) * 512],
                                     start=(j == 0), stop=(j == KT - 1))
                nc.vector.tensor_copy(v_t[:, p * C + n2 * 512:p * C + (n2 + 1) * 512],
                                      ps[:])

        p_t = ppool.tile([128, KT * LT], F16, tag="p")
        for h in range(H):
            pv_head(v_t, p_t, h)

        # output projection of the PREVIOUS strip (software pipelining: the
        # O-proj matmuls cover this strip's PV evictions and exp latency)
        o_proj(p_prev, ls_prev)
        p_prev, ls_prev = p_t, ls
    o_proj(p_prev, ls_prev, fine_tail=True)


def build_nc(l_total=L):
    nc = bacc.Bacc("TRN2", target_bir_lowering=False, debug=False,
                   enable_asserts=False)
    x_d = nc.dram_tensor("x", [C, l_total], F16, kind="ExternalInput").ap()
    w_d = {n: nc.dram_tensor(n, [C, C], F16, kind="ExternalInput").ap()
           for n in WNAMES}
    o_d = nc.dram_tensor("out", [C, l_total], F32, kind="ExternalOutput").ap()
    with tile.TileContext(nc) as tc:
        with ExitStack() as ctx:
            _emit(ctx, tc, x_d, w_d, o_d, l_total)
    nc.compile()
    return nc


_NC_CACHE = {}


def _get_nc(l_total):
    if l_total not in _NC_CACHE:
        _NC_CACHE[l_total] = build_nc(l_total)
    return _NC_CACHE[l_total]


def make_in_maps(x, Wq, Wk, Wv, Wo):
    x16 = np.ascontiguousarray(np.asarray(x).astype(np.float16))
    ws = {n: np.ascontiguousarray(np.asarray(w).astype(np.float16))
          for n, w in zip(WNAMES, (Wq, Wk, Wv, Wo))}
    in_maps = []
    for i in range(x.shape[0]):
        m = {"x": x16[i]}
        m.update(ws)
        in_maps.append(m)
    return in_maps


def _numpy_fallback(x, Wq, bq, Wk, bk, Wv, bv, Wo, bo):
    # Exact host-side path, used only if biases are nonzero (the problem spec
    # fills them with zeros, so the device kernel does not apply them).
    x = np.asarray(x, np.float32)
    Bn, Cn, Ln = x.shape
    hd = Cn // H
    nch = Ln // CHUNK
    xt = np.transpose(x, (0, 2, 1))
    Q = (xt @ Wq + bq).reshape(Bn, nch, CHUNK, H, hd)
    K = (xt @ Wk + bk).reshape(Bn, nch, CHUNK, H, hd)
    V = (xt @ Wv + bv).reshape(Bn, nch, CHUNK, H, hd)
    scores = np.einsum("bnqhd,bnkhd->bnhqk", Q, K) / np.sqrt(hd)
    scores -= scores.max(axis=-1, keepdims=True)
    e = np.exp(scores)
    attn = e / e.sum(axis=-1, keepdims=True)
    out = np.einsum("bnhqk,bnkhd->bnqhd", attn, V).reshape(Bn, Ln, Cn)
    out = out @ Wo + bo
    return np.ascontiguousarray(np.transpose(out, (0, 2, 1)).astype(np.float32))


def kernel(x, Wq, bq, Wk, bk, Wv, bv, Wo, bo, trace=False):
    from concourse.bass_utils import run_bass_kernel_spmd
    nb, c_in, l_total = x.shape
    if (any(np.any(np.asarray(b) != 0) for b in (bq, bk, bv, bo))
            or c_in != C or l_total % LT != 0 or nb > N_CORES):
        return _numpy_fallback(x, Wq, bq, Wk, bk, Wv, bv, Wo, bo)
    nc = _get_nc(l_total)
    in_maps = make_in_maps(x, Wq, Wk, Wv, Wo)
    res = run_bass_kernel_spmd(nc, in_maps, core_ids=list(range(nb)), trace=trace)
    out = np.stack([res.results[i]["out"] for i in range(nb)], axis=0)
    if trace:
        return out, res
    return out
